# revision 2
# baseline (speedup 1.0000x reference)
"""Trainium2 Bass kernel for a top-2 MoE layer (B=2, S=2048, H=1024, F=4096, E=8).

Strategy: expert-parallel across 8 NeuronCores. The (tiny) gate runs on host in
numpy; each core runs one expert's fc1->relu->fc2 over only the tokens routed to
it (padded to capacity C), in float32r (full-rate PE matmul, ~1e-4 matmul error).
Host scatter-adds the combine-weighted expert outputs back into the dense output.

Self-contained: hardcodes all shapes; imports only concourse + numpy.
"""

import sys
import types
import numpy as np
from contextlib import ExitStack

import concourse.bass as bass  # noqa: F401  (bass types used via tile/bacc)
import concourse.tile as tile
from concourse import bacc, mybir
from concourse.bass_utils import run_bass_kernel_spmd

B, S, H, F, E, K = 2, 2048, 1024, 4096, 8, 2
T = B * S
NH = H // 128   # 8 partition blocks of H
NF = F // 128   # 32 partition blocks of F
F32 = mybir.dt.float32
F32R = mybir.dt.float32r

_PROG_CACHE: dict = {}


def _token_chunks(tg):
    """Split a token span into moving-operand chunks (512s then one 256)."""
    out = []
    o = 0
    while o < tg:
        c = min(512, tg - o)
        out.append((o, c))
        o += c
    return out


def _build_program(C):
    """One expert's MLP over C tokens. Weights streamed in a single pass by
    splitting F into two halves; output accumulated in SBUF across halves."""
    nc = bacc.Bacc("TRN2", target_bir_lowering=False, debug=False, num_devices=E)

    xT = nc.dram_tensor("xT", [NH, 128, C], F32R, kind="ExternalInput").ap()
    w1d = nc.dram_tensor("w1d", [NF, 128, NH, 128], F32R, kind="ExternalInput").ap()
    w2d = nc.dram_tensor("w2d", [NH, 2, 128, NF // 2, 128], F32R, kind="ExternalInput").ap()
    b1d = nc.dram_tensor("b1d", [NF, 128, 1], F32, kind="ExternalInput").ap()
    yT = nc.dram_tensor("yT", [NH, 128, C], F32, kind="ExternalOutput").ap()

    chunks = _token_chunks(C)
    NFH = NF // 2  # f-blocks per half

    with tile.TileContext(nc) as tc, ExitStack() as ctx:
        xp = ctx.enter_context(tc.tile_pool(name="x", bufs=1))
        bp = ctx.enter_context(tc.tile_pool(name="b1", bufs=1))
        w1p = ctx.enter_context(tc.tile_pool(name="w1", bufs=3))
        w2p = ctx.enter_context(tc.tile_pool(name="w2", bufs=2))
        hp = ctx.enter_context(tc.tile_pool(name="h", bufs=1))
        oap = ctx.enter_context(tc.tile_pool(name="oacc", bufs=1))
        ps1p = ctx.enter_context(tc.tile_pool(name="ps1", bufs=3, space="PSUM"))
        ps2p = ctx.enter_context(tc.tile_pool(name="ps2", bufs=3, space="PSUM"))

        # Resident tokens (transposed) + biases + output accumulator
        xt = xp.tile([128, NH, C], F32R)
        for hb in range(NH):
            nc.sync.dma_start(xt[:, hb, :], xT[hb])
        b1t = bp.tile([128, NF, 1], F32)
        nc.sync.dma_start(b1t[:], b1d)
        oacc = oap.tile([128, NH, C], F32)

        for half in range(2):
            # hiddenT for this half: [128 (f part), NFH, C]
            ht = hp.tile([128, NFH, C], F32R, tag="ht")
            # --- fc1: hiddenT[fb] = relu(w1[:, fb].T @ x.T + b1[fb]) ---
            for fb in range(NFH):
                fbg = half * NFH + fb
                w1t = w1p.tile([128, NH, 128], F32R, tag="w1t")
                nc.sync.dma_start(w1t[:], w1d[fbg])
                for (o, ln) in chunks:
                    ps = ps1p.tile([128, 512], F32, tag="ps1")
                    for hb in range(NH):
                        nc.tensor.matmul(
                            ps[:, :ln],
                            w1t[:, hb, :],
                            xt[:, hb, o:o + ln],
                            start=(hb == 0),
                            stop=(hb == NH - 1),
                        )
                    nc.scalar.activation(
                        ht[:, fb, o:o + ln],
                        ps[:, :ln],
                        mybir.ActivationFunctionType.Relu,
                        bias=b1t[:, fbg, :],
                    )
            # --- fc2: yT[hb] (+)= w2[half, :, hb].T @ hiddenT ---
            for hb in range(NH):
                w2t = w2p.tile([128, NFH, 128], F32R, tag="w2t")
                nc.sync.dma_start(w2t[:], w2d[hb, half])
                for (o, ln) in chunks:
                    ps2 = ps2p.tile([128, 512], F32, tag="ps2")
                    for fb in range(NFH):
                        nc.tensor.matmul(
                            ps2[:, :ln],
                            w2t[:, fb, :],
                            ht[:, fb, o:o + ln],
                            start=(fb == 0),
                            stop=(fb == NFH - 1),
                        )
                    if half == 0:
                        nc.vector.tensor_copy(oacc[:, hb, o:o + ln], ps2[:, :ln])
                    else:
                        nc.vector.tensor_add(
                            oacc[:, hb, o:o + ln], oacc[:, hb, o:o + ln], ps2[:, :ln]
                        )
        for hb in range(NH):
            nc.sync.dma_start(yT[hb], oacc[:, hb, :])

    nc.compile()
    return nc


def _get_program(C):
    if C not in _PROG_CACHE:
        _PROG_CACHE[C] = _build_program(C)
    return _PROG_CACHE[C]


def kernel(hidden_states, gate_w, w1, b1, w2, b2):
    x = np.ascontiguousarray(np.asarray(hidden_states, np.float32).reshape(T, H))
    gw = np.asarray(gate_w, np.float32)
    w1 = np.asarray(w1, np.float32)
    b1 = np.asarray(b1, np.float32)
    w2 = np.asarray(w2, np.float32)
    b2 = np.asarray(b2, np.float32)

    # --- gate (host, replicates reference math) ---
    logits = (x @ gw).astype(np.float32)                   # [T, E]
    lm = logits.max(-1, keepdims=True)
    p = np.exp(logits - lm, dtype=np.float32)
    p = p / p.sum(-1, keepdims=True, dtype=np.float32)     # softmax [T, E]
    order = np.argsort(-p, axis=-1, kind="stable")
    topk_idx = order[:, :K]                                # [T, K]
    topk_p = np.take_along_axis(p, topk_idx, axis=-1)
    topk_p = topk_p / topk_p.sum(-1, keepdims=True)
    pm = p.mean(axis=0, dtype=np.float64)
    aux_loss = np.float32(E * np.sum(pm * pm))

    # per-expert token lists + combine weights
    idx_lists, cw_lists = [], []
    for e in range(E):
        rows, cols = np.nonzero(topk_idx == e)
        idx_lists.append(rows.astype(np.int64))
        cw_lists.append(topk_p[rows, cols].astype(np.float32))
    max_n = max(len(i) for i in idx_lists)
    C = max(256, -(-max_n // 256) * 256)

    nc = _get_program(C)

    in_maps = []
    for e in range(E):
        idx = idx_lists[e]
        pad = np.zeros(C, np.int64)
        pad[: len(idx)] = idx
        xg = x[pad]                                        # [C, H]
        xT_np = np.ascontiguousarray(xg.T).reshape(NH, 128, C)
        w1t = np.ascontiguousarray(
            w1[e].reshape(NH, 128, NF, 128).transpose(2, 1, 0, 3)
        )                                                  # [NF,128,NH,128]
        w2t = np.ascontiguousarray(
            w2[e].reshape(2, NF // 2, 128, NH, 128).transpose(3, 0, 2, 1, 4)
        )                                                  # [NH,2,128,NF/2,128]
        b1t = np.ascontiguousarray(b1[e].reshape(NF, 128, 1))
        in_maps.append({"xT": xT_np, "w1d": w1t, "w2d": w2t, "b1d": b1t})

    res = run_bass_kernel_spmd(nc, in_maps, list(range(E)))
    kernel.last_results = res

    out = np.zeros((T, H), np.float32)
    for e in range(E):
        n = len(idx_lists[e])
        y = res.results[e]["yT"].transpose(2, 0, 1).reshape(C, H)  # [C, H]
        out[idx_lists[e]] += cw_lists[e][:, None] * y[:n]
    # b2 term: reference adds b2 inside each expert, then weights by combine;
    # equivalent to adding sum_e c[t,e]*b2[e] here.
    cdense = np.zeros((T, E), np.float32)
    np.put_along_axis(cdense, topk_idx, topk_p, axis=-1)
    out += cdense @ b2
    return out.reshape(B, S, H), aux_loss
